# revision 3
# baseline (speedup 1.0000x reference)
import numpy as np
import jax
import jax.numpy as jnp

# nn_Attention_FishPP: hardcoded problem shapes
B, N, C = 64, 197, 768
H, GH, D = 12, 2, 64          # num_heads, global_heads, head_dim
HR = H // GH                  # 6
TOTAL_HEADS = 2 * GH + H      # 16
SCALE = D ** -0.5
LEVELS = 3
N_CORES = 8

ARG_ORDER = [
    "x", "qkv_w", "qkv_b", "masks", "mask_proj", "mask_base",
    "head_proj_w", "head_proj_b", "proj_w", "proj_b",
]


def _attn_shard(x, qkv_w, qkv_b, mw,
                head_proj_w, head_proj_b, proj_w, proj_b):
    # x: (B/8, N, C) shard; mw host-precomputed (1, GH, N, N, HR); rest replicated.
    b, n, c = x.shape
    qkv = (x @ qkv_w + qkv_b).reshape(b, n, TOTAL_HEADS, D).transpose(0, 2, 1, 3)
    q = qkv[:, :GH]
    k = qkv[:, GH:2 * GH]
    v = qkv[:, 2 * GH:]

    attn = jnp.einsum("bgnd,bgmd->bgnm", q, k) * SCALE

    a = attn[..., None] * mw                                # (b, gh, n, n, hr)
    a = a.transpose(0, 2, 3, 1, 4).reshape(b, n, n, H)
    a = jax.nn.relu(a) @ head_proj_w + head_proj_b
    a = a.transpose(0, 3, 1, 2)                             # (b, h, n, n)
    a = jax.nn.softmax(a, axis=-1)
    out = jnp.einsum("bhnm,bhmd->bnhd", a, v).reshape(b, n, c)
    return out @ proj_w + proj_b


_compiled = None


def _get_compiled():
    global _compiled
    if _compiled is None:
        _compiled = jax.pmap(
            _attn_shard,
            axis_name="x",
            in_axes=(0,) + (None,) * 7,
            devices=jax.devices()[:N_CORES],
        )
    return _compiled


def kernel(**inputs: np.ndarray) -> np.ndarray:
    fn = _get_compiled()
    x = np.ascontiguousarray(inputs["x"], dtype=np.float32)
    x_sh = x.reshape(N_CORES, B // N_CORES, N, C)

    # host-side precompute of the per-pair mask weights (tiny: N*N*H)
    masks = np.asarray(inputs["masks"], dtype=np.float32)
    mw = masks.reshape(N * N, LEVELS) @ np.asarray(inputs["mask_proj"], np.float32)
    mw += np.asarray(inputs["mask_base"], np.float32)
    mw = np.ascontiguousarray(
        mw.reshape(N, N, GH, HR).transpose(2, 0, 1, 3)[None]
    )  # (1, GH, N, N, HR)

    args = [
        x_sh,
        np.asarray(inputs["qkv_w"], np.float32),
        np.asarray(inputs["qkv_b"], np.float32),
        mw,
        np.asarray(inputs["head_proj_w"], np.float32),
        np.asarray(inputs["head_proj_b"], np.float32),
        np.asarray(inputs["proj_w"], np.float32),
        np.asarray(inputs["proj_b"], np.float32),
    ]
    with jax.default_matmul_precision("highest"):
        out = fn(*args)
    out = np.asarray(out, dtype=np.float32).reshape(B, N, C)
    return out
